# revision 1
# baseline (speedup 1.0000x reference)
"""Trainium2 Bass kernel for sparse (Minkowski) voxel convolution.

out[i] = sum_k mask[k,i] * features[in_map[k,i]] @ W[k]
  features [N=100000, C=128] f32, W [K=27, 128, 128] f32,
  in_map/valid_mask [27, N].

Strategy (8 NeuronCores, SPMD, no collectives):
  * Shard output rows across cores (12500/core, padded to 12800).
  * Each core processes its points in rounds. For each round, the host
    builds a compact bf16 feature table containing only the rows that
    round references (plus a zero row at index 0 for masked/padded
    entries), so gather indices fit in int16 — the requirement of the
    HW `dma_gather` instruction.
  * On device, per (round, k): one dma_gather(transpose=True) pulls the
    gathered rows directly into SBUF in [C, pts] layout (the DMA xbar
    does the transpose), then the tensor engine accumulates
    psum[:, tile] += W[k].T @ G_k.T over all 27 offsets in fp32 PSUM.
  * Output is written as out.T [128, 12800] f32 per core; the host
    transposes/unpads and concatenates. bf16 inputs + fp32 accumulation
    keep relative error ~2e-3.
"""

import sys

for _p in ("/opt/trn_rl_repo", "/root/.axon_site/_ro/trn_rl_repo"):
    if _p not in sys.path:
        sys.path.insert(0, _p)

import numpy as np
import ml_dtypes

N = 100000
C = 128
K = 27
NCORES = 8
PTS_PER_CORE = N // NCORES          # 12500
PADDED_PTS = 12800                  # per-core, multiple of 2560/1280/640
TABLE_ROWS = 32768                  # int16 index limit
MAX_MM_FREE = 512                   # one fp32 PSUM bank
MAX_GATHER = 896                    # descs per dma_gather; ring holds ~1k


def _point_tiles(round_pts):
    """Split a round into matmul tiles (<=512, multiple of 128)."""
    tiles = []
    off = 0
    while off < round_pts:
        t = min(MAX_MM_FREE, round_pts - off)
        assert t % 128 == 0
        tiles.append((off, t))
        off += t
    return tiles


def _build_program(round_pts, rounds, table_rows=TABLE_ROWS, iters=1,
                   g_bufs=6, n_queues=1, single_packet=False):
    # single_packet=False is the key performance lever: with the default
    # (True) each gather's whole descriptor stream is one packet consumed
    # by a single SDMA engine (~20.8 ns/row, serial). Independent per-row
    # packets spread across all 16 engines (~3x faster end-to-end).
    # NOTE: n_queues>1 produced wrong results at full 8-core scale.
    """Build the per-core Bass program (SPMD: same program, all cores)."""
    import concourse.bacc as bacc
    import concourse.mybir as mybir
    import concourse.tile as tile

    idx_cols_per_rk = round_pts // 16
    idx_cols = rounds * K * idx_cols_per_rk
    n_pts = rounds * round_pts

    nc = bacc.Bacc("TRN2", target_bir_lowering=False, debug=False,
                   num_swdge_queues=n_queues)
    table_d = nc.dram_tensor(
        "table", [rounds, table_rows, C], mybir.dt.bfloat16, kind="ExternalInput")
    idx_d = nc.dram_tensor(
        "idx", [128, idx_cols], mybir.dt.int16, kind="ExternalInput")
    wmat_d = nc.dram_tensor(
        "wmat", [C, K * C], mybir.dt.bfloat16, kind="ExternalInput")
    out_d = nc.dram_tensor(
        "out_t", [C, n_pts], mybir.dt.float32, kind="ExternalOutput")

    tiles = _point_tiles(round_pts)

    with tile.TileContext(nc) as tc:
        with (
            tc.tile_pool(name="const", bufs=1) as cpool,
            tc.tile_pool(name="g", bufs=g_bufs) as gpool,
            tc.tile_pool(name="ostage", bufs=4) as opool,
            tc.tile_pool(name="psum", bufs=8, space="PSUM") as ppool,
        ):
            w_sb = cpool.tile([C, K * C], mybir.dt.bfloat16)
            idx_sb = cpool.tile([128, idx_cols], mybir.dt.int16)
            nc.sync.dma_start(w_sb[:], wmat_d.ap())
            nc.sync.dma_start(idx_sb[:], idx_d.ap())

            def body(_iv=None):
                for r in range(rounds):
                    ps = []
                    for (off, tl) in tiles:
                        p = ppool.tile([C, tl], mybir.dt.float32,
                                       name=f"ps_r{r}_{off}", tag="ps")
                        ps.append(p)
                    for k in range(K):
                        g = gpool.tile([128, 1, round_pts], mybir.dt.bfloat16,
                                       name=f"g_r{r}_k{k}", tag="g")
                        col = (r * K + k) * idx_cols_per_rk
                        # Sub-gathers: the SWDGE descriptor ring holds ~1k
                        # descriptors and one gather's whole stream must fit,
                        # so cap indices per dma_gather. Entry j of the round
                        # lives at idx[j % 16, col + j // 16], so a sub-range
                        # [o, o+S) is the column slice [col+o/16, col+(o+S)/16).
                        o = 0
                        while o < round_pts:
                            s = min(MAX_GATHER, round_pts - o)
                            nc.gpsimd.dma_gather(
                                g[:, :, o:o + s],
                                table_d.ap()[r],
                                idx_sb[:, col + o // 16:col + (o + s) // 16],
                                s,
                                s,
                                C,
                                transpose=True,
                                queue_num=(r * K + k) % n_queues,
                                single_packet=single_packet,
                            )
                            o += s
                        for ti, (off, tl) in enumerate(tiles):
                            nc.tensor.matmul(
                                ps[ti][:],
                                w_sb[:, k * C:(k + 1) * C],
                                g[:, 0, off:off + tl],
                                start=(k == 0),
                                stop=(k == K - 1),
                            )
                    for ti, (off, tl) in enumerate(tiles):
                        o = opool.tile([C, tl], mybir.dt.float32,
                                       name=f"o_r{r}_{off}", tag="o")
                        nc.vector.tensor_copy(o[:], ps[ti][:])
                        base = r * round_pts + off
                        nc.sync.dma_start(out_d.ap()[:, base:base + tl], o[:])

            if iters == 1:
                body()
            else:
                with tc.For_i(0, iters, 1):
                    body()
    nc.compile()
    return nc


def _prep_core_inputs(F_bf, W_flat, in_map, valid_mask, pts, round_pts, rounds,
                      table_rows=TABLE_ROWS):
    """Host-side compaction for one core.

    pts: global point ids owned by this core (len <= rounds*round_pts;
    tail is padded with masked dummy points).
    Returns the in_map dict for run_bass_kernel_spmd.
    """
    npts_pad = rounds * round_pts
    idx_cols_per_rk = round_pts // 16
    table = np.zeros((rounds, table_rows, C), dtype=ml_dtypes.bfloat16)
    idx = np.zeros((128, rounds * K * idx_cols_per_rk), dtype=np.int16)

    im = in_map[:, pts]                      # [K, npts_real]
    vm = valid_mask[:, pts]
    npts_real = len(pts)

    for r in range(rounds):
        lo = r * round_pts
        hi = min(lo + round_pts, npts_real)
        if lo >= npts_real:
            continue
        im_r = im[:, lo:hi]
        vm_r = vm[:, lo:hi]
        uniq = np.unique(im_r[vm_r])
        if len(uniq) + 1 > table_rows:
            raise OverflowError(f"round table overflow: {len(uniq)+1}")
        table[r, 1:1 + len(uniq)] = F_bf[uniq]
        idx16 = np.where(vm_r, 1 + np.searchsorted(uniq, im_r), 0).astype(np.int16)
        if hi - lo < round_pts:                       # pad tail points
            idx16 = np.pad(idx16, ((0, 0), (0, round_pts - (hi - lo))))
        # dma_gather index layout: entry j lives at [j % 16, j // 16],
        # and the 16-partition block must be replicated across all eight
        # 16-partition stripes (each GPSIMD Q7 core streams indices from
        # its own stripe; TX descriptors come from a core whose stripe
        # is not partitions 0-15).
        blk = idx16.reshape(K, idx_cols_per_rk, 16)   # [K, s, p]
        for k in range(K):
            c0 = (r * K + k) * idx_cols_per_rk
            idx[:, c0:c0 + idx_cols_per_rk] = np.tile(blk[k].T, (8, 1))
    return {"table": table, "idx": idx, "wmat": W_flat}


def _choose_round_pts(in_map, valid_mask, table_rows=TABLE_ROWS):
    for round_pts in (2560, 1280, 640):
        rounds = PADDED_PTS // round_pts
        ok = True
        for c in range(NCORES):
            base = c * PTS_PER_CORE
            for r in range(rounds):
                lo = base + r * round_pts
                hi = min(lo + round_pts, base + PTS_PER_CORE)
                if lo >= hi:
                    continue
                sel = valid_mask[:, lo:hi]
                n_uniq = len(np.unique(in_map[:, lo:hi][sel]))
                if n_uniq + 1 > table_rows:
                    ok = False
                    break
            if not ok:
                break
        if ok:
            return round_pts, rounds
    raise OverflowError("no feasible round size")


def kernel(features, kernel, in_map, valid_mask):
    from concourse import bass_utils

    F = np.asarray(features, dtype=np.float32)
    W = np.asarray(kernel, dtype=np.float32)
    im = np.asarray(in_map, dtype=np.int32)
    vm = np.asarray(valid_mask, dtype=bool)
    assert F.shape == (N, C) and W.shape == (K, C, C)

    F_bf = F.astype(ml_dtypes.bfloat16)
    # wmat[ci, k*C+co] = W[k, ci, co]  (lhsT layout, per-offset stationary)
    W_flat = np.ascontiguousarray(
        np.transpose(W, (1, 0, 2)).reshape(C, K * C)).astype(ml_dtypes.bfloat16)

    round_pts, rounds = _choose_round_pts(im, vm)
    nc = _build_program(round_pts, rounds)

    in_maps = []
    for c in range(NCORES):
        pts = np.arange(c * PTS_PER_CORE, (c + 1) * PTS_PER_CORE)
        in_maps.append(
            _prep_core_inputs(F_bf, W_flat, im, vm, pts, round_pts, rounds))

    res = bass_utils.run_bass_kernel_spmd(
        nc, in_maps, core_ids=list(range(NCORES)))

    out = np.empty((N, C), dtype=np.float32)
    for c in range(NCORES):
        o = res.results[c]["out_t"]          # [C, rounds*round_pts]
        out[c * PTS_PER_CORE:(c + 1) * PTS_PER_CORE] = o[:, :PTS_PER_CORE].T
    return out

